# revision 34
# baseline (speedup 1.0000x reference)
"""Trainium2 Bass kernel for nn_LHFA_76278619177511.

Fused transposed-attention block (LHFA):
  q = dwconv3(conv1x1(x, Wq), Wq_dw)   (k from y, v from z)
  attn = softmax(l2norm(q) @ l2norm(k)^T * temp)   per-head [32,32]
  out = Wproj @ (attn @ v) + Wfus @ [x;y;z]

V2 strategy (pure DP over batch B=8 on 8 cores), key ideas:
  - fp8 DoubleRow matmuls (0.5 cycles/row, 2 K-tiles per instruction):
    the merged 1x1+dw conv (K=576) runs in 3 DR matmuls instead of 5
    bf16 passes. Pad row pitch = 144 so every DR pair stride/offset is
    16-aligned (dual-fp8 Ldweights ISA restriction).
  - q/k path entirely in fp8-e4m3 with weights pre-scaled x128; the
    scale cancels exactly in the L2 normalization.
  - The whole v path is folded into weights: W2T = (Wproj@attn@V9)^T is
    built on-device with tiny matmuls after softmax, then the attention
    output term becomes a single fused conv W2T^T @ z9 in fp8-e5m2
    accumulated INTO THE SAME PSUM as the bf16 fus conv (scales cancel:
    W2T stored x2^8, z fed x2^-8). No v slab, no attn@v, no proj.
  - Grams/sq-norms via DR on row pairs (64 pairs over the image).
  - attn output term is ~500x smaller than the fus term, so fp8 noise
    in it is invisible at the 2e-2 gate; fus stays bf16.
"""

import numpy as np
import ml_dtypes

import bass_rust
import concourse.bass as bass
import concourse.mybir as mybir
from concourse import tile as tile_mod
from concourse.tile import TileContext
from concourse.vector_clock import ScopedClock
from concourse.bass_utils import run_bass_kernel_spmd

BF16 = mybir.dt.bfloat16
F32 = mybir.dt.float32
E4 = mybir.dt.float8e4
E5 = mybir.dt.float8e5
NP_E4 = ml_dtypes.float8_e4m3
NP_E5 = ml_dtypes.float8_e5m2
NP_BF = ml_dtypes.bfloat16
DR = mybir.MatmulPerfMode.DoubleRow

C = 64          # input channels
DIM = 256       # q/k/v channels
HEADS = 8
H = W = 128
N = H * W       # 16384
PW = 144        # padded row pitch (16-aligned; image cols at [1,129))
HB = 16         # band height
NB = H // HB    # 8 bands
NRT = HB + 2    # rows per band tile
TWA = NRT * PW  # 2592: region width (AB / AD each)
ADB = 2606      # AD region base col (== 14 mod 16 so P1 stride is 16-mult)
FA = 7168       # pad tile alloc width (slice-bound slack)

# DR pair definitions: (X0(base), delta). base = hl*PW.
#   P0: (S0 @ AB+base,        S1 @ AB+base+PW)        taps rows -1, 0
#   P1: (S2 @ AB+base+2PW,    S3 @ AD+base+2)         taps row +1, col +1
#   P2: (dummy @ AD+base+2,   S4 @ AD+base+2PW+2)     tap (1,1) (A only)
P0D = PW
P1D = ADB + 2 - 2 * PW   # 2320
P2D = 2 * PW             # 288
assert P1D % 16 == 0 and ADB % 16 == 14
# weight slots [6 x 256]: 0=S0 1=S1 2=S2 3=S3 4=ZERO 5=S4
# taps (dy,dx) per slot/half for weight merging (half 0 = A rows,
# half 1 = B rows (slots 0-2, B = A<<1col) or D rows (slot 3, D = A<<1row))
SLOT_TAPS = [
    [(-1, -1), (-1, 0)],
    [(0, -1), (0, 0)],
    [(1, -1), (1, 0)],
    [(-1, 1), (0, 1)],
    [],
    [(1, 1)],
]

QK_SCALE = 128.0      # q/k conv weight prescale (cancels in l2norm)
V9_SCALE = 2.0 ** 12  # V9 prescale into e4m3
PR_SCALE = 2.0 ** 6   # WprojN prescale into e4m3
AT_SCALE = 32.0       # attn (Pt) prescale into e4m3
W2_OUT_SCALE = 2.0 ** 8   # W2T stored scale (z fed at 2^-8)
Z8_SCALE = 2.0 ** -8
# W2 psum carries AT*V9*PR = 2^23; store at 2^8 -> copy scale 2^-15
W2_COPY_SCALE = W2_OUT_SCALE / (AT_SCALE * V9_SCALE * PR_SCALE)

_PATCHED = False


def _patch_tile_drain():
    """This walrus build rejects >1 sem wait on a CTRL (Drain) instruction;
    split the TileContext tail-drain waits onto individual nops."""
    global _PATCHED
    if _PATCHED:
        return
    _PATCHED = True

    def _drain_and_barrier(self, tick_clock, wait_clock):
        nc = self.nc
        drain_inst = nc.sync.drain()
        wait_clock.add_sem_waits(
            drain_inst.ins, ScopedClock({None: tick_clock.global_clock})
        )
        si = drain_inst.ins.sync_info
        waits = list(si.on_wait or [])
        if len(waits) > 1:
            si.on_wait = waits[:1]
            for w in waits[1:]:
                nop = nc.sync.nop(nofuse=True, hint="split_wait")
                nop.ins.sync_info = bass_rust.SyncInfo(on_wait=[w], on_update=[])
        nc.all_engine_barrier()
        assert self.sems is not None
        popped = nc._tile_sem_poison_stack.pop()
        assert popped is self._sem_poison
        nc.clear_and_free_semaphores(list(self.sems.allocated().values()))
        nc.all_engine_barrier()

    tile_mod.TileContext._drain_and_barrier = _drain_and_barrier
    try:
        from concourse import tile_utils
        tile_utils.max_sbuf_usage = 208 * 1024
    except Exception:
        pass


def _split_excess_waits(nc, max_waits=1):
    """This walrus build caps sem waits per instruction encoding; hoist
    excess waits onto preceding same-engine NoOps (queues are in-order,
    so a wait satisfied on an earlier instruction orders the later one)."""
    import bass_rust as _br

    ctr = [0]
    for f in nc.m.functions:
        for blk in f.blocks:
            out = []
            for inst in blk.instructions:
                si = inst.sync_info
                waits = list(si.on_wait) if (si and si.on_wait) else []
                if len(waits) > max_waits:
                    keep = waits[:max_waits]
                    extra = waits[max_waits:]
                    si.on_wait = keep
                    for w in extra:
                        ctr[0] += 1
                        nop = _br.InstNoOp(name=f"wsplit-{ctr[0]}", ins=[], outs=[])
                        nop.engine = inst.engine
                        nop.sync_info = _br.SyncInfo(on_wait=[w], on_update=[])
                        try:
                            nc.register_instruction(nop, overwrite=True)
                        except Exception:
                            pass
                        out.append(nop)
                out.append(inst)
            blk.instructions[:] = out


def _merge_w(W1, Wdw, scale):
    """-> [128, 6, 256] float32 merged conv weights in slot layout.
    [p=(half,chan), slot, outch] = dw[out, tap(slot, half)] * W1[out, chan]."""
    out = np.zeros((128, 6, 256), np.float32)
    W1 = W1[:, :, 0, 0]  # [256, 64]
    for s, taps in enumerate(SLOT_TAPS):
        for half, (dy, dx) in enumerate(taps):
            hh = half if s != 5 else 0
            out[hh * 64: (hh + 1) * 64, s, :] = (
                Wdw[:, 0, 1 + dy, 1 + dx][:, None] * W1
            ).T * scale
    return out


def _merge_v9(W1, Wdw, scale):
    """-> [256, 576] float32: V9[d, s*128 + half*64 + c] for slots 0-3 +
    slot5(A half) packed as k-chunk order s in 0..4 (chunk4 = slot5 taps)."""
    out = np.zeros((256, 640), np.float32)
    W1 = W1[:, :, 0, 0]  # [256, 64]
    for s, taps in enumerate(SLOT_TAPS):
        if s == 4:
            continue
        kc = s if s < 4 else 4
        for half, (dy, dx) in enumerate(taps):
            hh = half if s != 5 else 0
            out[:, kc * 128 + hh * 64: kc * 128 + (hh + 1) * 64] = (
                Wdw[:, 0, 1 + dy, 1 + dx][:, None] * W1
            ) * scale
    return out[:, :576]


def _bf(a):
    return np.ascontiguousarray(a).astype(NP_BF)


def _pair_ap(tile_ap, X, delta, width):
    """[128, 2, width] AP: members at cols X and X+delta."""
    return tile_ap[:, X: X + 2 * delta].rearrange(
        "p (two m) -> p two m", two=2
    )[:, :, 0:width]


def _build_nc(wq, wk, v9, wprojN, wfusT, temp_cols):
    """Build the Bass module. Weight arrays pre-merged/scaled fp32."""
    _patch_tile_drain()
    nc = bass.Bass()

    # xy interleaved for the fus loads: parts 0:64 = x, 64:128 = y
    xyd = nc.declare_dram_parameter("xy", [128, N], BF16, isOutput=False)
    zd = nc.declare_dram_parameter("z", [C, N], BF16, isOutput=False)
    # pre-padded, pre-shifted canvases [128, 2*LC]: cols 0:LC = AB
    # (parts 0:64 = A padded image, 64:128 = B = A<<1col), cols LC:2LC =
    # AD (A | D = A<<1row)
    LC = 130 * PW
    pad_d = {}
    for nm, dt8 in (("xc", E4), ("yc", E4), ("zc", E5)):
        pad_d[nm] = nc.declare_dram_parameter(
            nm, [128, 2 * LC], dt8, isOutput=False)
    od = nc.declare_dram_parameter("out", [DIM, N], BF16, isOutput=True)

    wq_d = nc.inline_tensor(
        np.ascontiguousarray(wq.reshape(128, 6 * 256)).astype(NP_E4), name="wq9")
    wk_d = nc.inline_tensor(
        np.ascontiguousarray(wk.reshape(128, 6 * 256)).astype(NP_E4), name="wk9")
    # V9 [256,576] -> 2 mb tiles side by side [128, 1152]
    v9_2 = np.concatenate([v9[0:128], v9[128:256]], axis=1)
    v9_d = nc.inline_tensor(np.ascontiguousarray(v9_2).astype(NP_E4), name="v9")
    wp_d = nc.inline_tensor(
        np.ascontiguousarray(wprojN).astype(NP_E4), name="wprojN")  # [128, 512]
    wf_d = nc.inline_tensor(_bf(wfusT), name="wfusT")               # [128, 512]
    tc0_d = nc.inline_tensor(np.ascontiguousarray(temp_cols[0]), name="tcol0")
    tc1_d = nc.inline_tensor(np.ascontiguousarray(temp_cols[1]), name="tcol1")
    id_d = nc.inline_tensor(np.eye(128, dtype=NP_BF), name="ident")

    with TileContext(nc) as tc:
        import contextlib

        with contextlib.ExitStack() as ctx:
            wpool = ctx.enter_context(tc.tile_pool(name="wpool", bufs=1))
            pads = ctx.enter_context(tc.tile_pool(name="pads", bufs=3))
            qkp = ctx.enter_context(tc.tile_pool(name="qkp", bufs=3))
            smallp = ctx.enter_context(tc.tile_pool(name="smallp", bufs=2))

            # --- weights to SBUF ---
            wq_sb = wpool.tile([128, 6 * 256], E4, tag="wq")
            wk_sb = wpool.tile([128, 6 * 256], E4, tag="wk")
            v9_sb = wpool.tile([128, 2 * 576], E4, tag="v9")
            wp_sb = wpool.tile([128, 512], E4, tag="wp")
            wf_sb = wpool.tile([128, 512], BF16, tag="wf")
            # w2t as 3 separate pair tiles so phase-2 DRs only wait on the
            # slots they read
            w2t_t = [wpool.tile([128, 512], E5, tag=f"w2t{k}",
                                name=f"w2t{k}") for k in range(3)]
            ident_sb = wpool.tile([128, 128], BF16, tag="ident")
            tcol = [wpool.tile([128, 1], F32, tag=f"tc{i}", name=f"tcol{i}")
                    for i in range(2)]

            # --- pass-1 psums (qk pool created last, closes first: LIFO) ---
            p1stack = ctx.enter_context(contextlib.ExitStack())
            qkstack = ctx.enter_context(contextlib.ExitStack())
            ps_acc = p1stack.enter_context(
                tc.tile_pool(name="ps_acc", bufs=1, space="PSUM"))
            ps_qk = qkstack.enter_context(
                tc.tile_pool(name="ps_qk", bufs=3, space="PSUM"))
            acc1 = ps_acc.tile([128, 512], F32, tag="acc1")
            acc2 = ps_acc.tile([128, 256], F32, tag="acc2")
            par_all = acc1[:, 0:256]
            pgq = acc1[:, 256:512]
            pgk = acc2

            def build_pads(pool, dc, name, band, dt8):
                """Load the [128, FA] padded AB|AD tile for one band in ONE
                two-range DMA from the host-baked shifted canvas."""
                P = pool.tile([128, FA], dt8, tag=f"pad{name}")
                o = (HB * band) * PW
                src = dc[:].rearrange(
                    "p (two l) -> p two l", l=LC)[:, :, o: o + TWA]
                dst = P[:][:, 0: 2 * ADB].rearrange(
                    "p (two w) -> p two w", two=2)[:, :, 0:TWA]
                nc.sync.dma_start(out=dst, in_=src)
                return P

            # band-0 pads first so the first conv's data leads the DMA
            # device queue; head rows split out so row-0 convs start early
            def build_pads_split(dc, name, dt8, hr):
                P = pads.tile([128, FA], dt8, tag=f"pad{name}")
                src3 = dc[:].rearrange("p (two l) -> p two l", l=LC)
                dst3 = P[:][:, 0: 2 * ADB].rearrange(
                    "p (two w) -> p two w", two=2)
                cut = hr * PW
                nc.sync.dma_start(out=dst3[:, :, 0:cut],
                                  in_=src3[:, :, 0:cut])
                nc.sync.dma_start(out=dst3[:, :, cut:TWA],
                                  in_=src3[:, :, cut:TWA])
                return P

            xP0 = build_pads_split(pad_d["xc"], "x", E4, 6)
            nc.scalar.dma_start(out=wq_sb, in_=wq_d[:])
            yP0 = build_pads_split(pad_d["yc"], "y", E4, 6)
            nc.scalar.dma_start(out=wk_sb, in_=wk_d[:])
            nc.gpsimd.dma_start(out=v9_sb, in_=v9_d[:])
            nc.gpsimd.dma_start(out=wp_sb, in_=wp_d[:])
            nc.gpsimd.dma_start(out=wf_sb, in_=wf_d[:])
            nc.gpsimd.dma_start(out=tcol[0], in_=tc0_d[:])
            nc.gpsimd.dma_start(out=tcol[1], in_=tc1_d[:])
            nc.gpsimd.dma_start(out=ident_sb, in_=id_d[:])

            def conv_drs(P, w_sb, pt, base, start_tag):
                """3 DR matmuls accumulating one row's conv into pt [128,256]."""
                ap = P[:]
                w3 = w_sb[:].rearrange("p (s n) -> p s n", n=256)
                pairs = [
                    (base, P0D, 0),
                    (base + 2 * PW, P1D, 2),
                    (ADB + base + 2, P2D, 4),
                ]
                for j, (X, D_, ws) in enumerate(pairs):
                    nc.tensor.matmul(
                        pt,
                        lhsT=_pair_ap(ap, X, D_, 128),
                        rhs=w3[:, ws: ws + 2, :],
                        start=(j == 0),
                        stop=(j == 2),
                        perf_mode=DR,
                    )

            # ================= pass 1: q/k convs + grams =================
            pend_gram = None  # (cat tile, first, last)

            def emit_grams(cat, first, last):
                cat3 = cat[:].rearrange("p (two c) -> p two c", two=2)
                for mb in range(2):
                    qsl = cat3[:, :, 256 + 128 * mb: 256 + 128 * mb + 128]
                    ksl = cat3[:, :, 128 * mb: 128 * mb + 128]
                    nc.tensor.matmul(
                        par_all[:, bass.ds(mb * 128, 128)],
                        lhsT=qsl, rhs=ksl, start=first, stop=last,
                        perf_mode=DR, skip_group_check=True)
                    nc.tensor.matmul(
                        pgq[:, bass.ds(mb * 128, 128)],
                        lhsT=qsl, rhs=qsl, start=first, stop=last,
                        perf_mode=DR, skip_group_check=True)
                    nc.tensor.matmul(
                        pgk[:, bass.ds(mb * 128, 128)],
                        lhsT=ksl, rhs=ksl, start=first, stop=last,
                        perf_mode=DR, skip_group_check=True)

            # persistent fus-input slabs, preloaded during pass 1 (the DMA
            # device has slack there; phase 2 then only moves pads + output)
            fuspool = ctx.enter_context(tc.tile_pool(name="fusp", bufs=1))
            xy_slab = fuspool.tile([128, N], BF16, tag="xyslab", name="xyslab")
            z_slab = fuspool.tile([64, N], BF16, tag="zslab", name="zslab")

            for b in range(NB):
                xP = xP0 if b == 0 else build_pads(
                    pads, pad_d["xc"], "x", b, E4)
                yP = yP0 if b == 0 else build_pads(
                    pads, pad_d["yc"], "y", b, E4)
                n0 = b * (N // NB)
                nc.gpsimd.dma_start(
                    out=xy_slab[:, bass.ds(n0, N // NB)],
                    in_=xyd[:, bass.ds(n0, N // NB)])
                nc.gpsimd.dma_start(
                    out=z_slab[:, bass.ds(n0, N // NB)],
                    in_=zd[:, bass.ds(n0, N // NB)])
                for hl2 in range(HB // 2):
                    pqk = ps_qk.tile([128, 1024], F32, tag="pqk")
                    for half in range(2):  # even/odd row of the pair
                        base = (2 * hl2 + half) * PW
                        o = 512 * half
                        conv_drs(yP, wk_sb, pqk[:, o: o + 256], base, "k")
                        conv_drs(xP, wq_sb, pqk[:, o + 256: o + 512], base, "q")
                    cat = qkp.tile([128, 1024], E4, tag="cat")
                    nc.scalar.copy(cat[:, 0:512], pqk[:, 0:512])
                    nc.vector.tensor_copy(cat[:, 512:1024], pqk[:, 512:1024])
                    if pend_gram is not None:
                        emit_grams(*pend_gram)
                    pg = 8 * b + hl2
                    pend_gram = (cat, pg == 0, pg == 63)
            emit_grams(*pend_gram)
            qkstack.close()
            # prefetch z band-0 pads early (DMA only; overlaps pass-1 tail)
            zP0 = build_pads(pads, pad_d["zc"], "z", 0, E5)

            # --- phase-2 pools + prefill: fus matmuls run on the PE while
            # the softmax chain occupies ACT/DVE ---
            p2p = ctx.enter_context(tc.tile_pool(name="p2p", bufs=4))
            ps_o = ctx.enter_context(
                tc.tile_pool(name="ps_o", bufs=6, space="PSUM"))

            def p2_fus(g):
                n0 = 512 * g
                pos = []
                for mb in range(2):
                    po = ps_o.tile([128, 512], F32, tag="po")
                    nc.tensor.matmul(
                        po, lhsT=wf_sb[:, bass.ds(mb * 128, 128)],
                        rhs=xy_slab[:, bass.ds(n0, 512)],
                        start=True, stop=False)
                    nc.tensor.matmul(
                        po, lhsT=wf_sb[0:64, bass.ds(256 + mb * 128, 128)],
                        rhs=z_slab[:, bass.ds(n0, 512)],
                        start=False, stop=False)
                    pos.append(po)
                return pos

            # slot 4 (pair tile 2, first half) is zeros; k-chunk 4 -> slot 5
            nc.gpsimd.memset(w2t_t[2][:, 0:256], 0.0)
            nc.gpsimd.memset(w2t_t[2][64:128, 256:512], 0.0)
            prefill = {g: p2_fus(g) for g in range(3)}

            # ================= softmax on per-head [32,32] =================
            bd8 = [smallp.tile([128, 128], E4, tag=f"bd{mb}",
                               name=f"bdiag{mb}") for mb in range(2)]
            for mb in range(2):
                scr = smallp.tile([128, 128], F32, tag="scr")
                rnq_c = smallp.tile([128, 1], F32, tag="rnq")
                rnk_c = smallp.tile([128, 1], F32, tag="rnk")
                for g_ps, dst in ((pgq, rnq_c), (pgk, rnk_c)):
                    ssum = smallp.tile([128, 1], F32, tag="ssum")
                    nc.vector.tensor_mul(
                        scr, g_ps[:, bass.ds(mb * 128, 128)], ident_sb)
                    nc.vector.reduce_sum(
                        out=ssum, in_=scr, axis=mybir.AxisListType.X)
                    nc.scalar.sqrt(ssum, ssum)
                    nc.vector.tensor_scalar_max(ssum, ssum, 1e-12)
                    nc.vector.reciprocal(dst, ssum)
                rnqt = smallp.tile([128, 1], F32, tag="rnqt")
                nc.vector.tensor_mul(rnqt, rnq_c, tcol[mb])

                hd = smallp.tile([128, 32], F32, tag="hd")
                for i in range(4):
                    nc.vector.tensor_copy(
                        hd[32 * i: 32 * (i + 1), :],
                        par_all[32 * i: 32 * (i + 1),
                                bass.ds(mb * 128 + 32 * i, 32)],
                    )
                hds = smallp.tile([128, 32], F32, tag="hds")
                nc.scalar.activation(
                    hds, hd, mybir.ActivationFunctionType.Copy,
                    bias=0.0, scale=rnqt)
                hdT = smallp.tile([128, 32], F32, tag="hdT")
                nc.vector.transpose(hdT, hds)
                hdTs = smallp.tile([128, 32], F32, tag="hdTs")
                nc.scalar.activation(
                    hdTs, hdT, mybir.ActivationFunctionType.Copy,
                    bias=0.0, scale=rnk_c)
                hd3 = smallp.tile([128, 32], F32, tag="hd3")
                nc.vector.transpose(hd3, hdTs)
                nmx = smallp.tile([128, 1], F32, tag="nmx")
                nc.vector.reduce_max(
                    out=nmx, in_=hd3, axis=mybir.AxisListType.X, negate=True)
                ex = smallp.tile([128, 32], F32, tag="ex")
                nc.scalar.activation(
                    ex, hd3, mybir.ActivationFunctionType.Exp,
                    bias=nmx, scale=1.0)
                sm = smallp.tile([128, 1], F32, tag="sm")
                nc.vector.reduce_sum(out=sm, in_=ex, axis=mybir.AxisListType.X)
                rsm = smallp.tile([128, 1], F32, tag="rsm")
                nc.vector.reciprocal(rsm, sm)
                rsm32 = smallp.tile([128, 1], F32, tag="rsm32")
                nc.scalar.activation(
                    rsm32, rsm, mybir.ActivationFunctionType.Copy,
                    bias=0.0, scale=AT_SCALE)
                Pt = smallp.tile([128, 32], F32, tag="Pt")
                nc.scalar.activation(
                    Pt, ex, mybir.ActivationFunctionType.Copy,
                    bias=0.0, scale=rsm32)
                PtT = smallp.tile([128, 32], F32, tag="PtT")
                nc.vector.transpose(PtT, Pt)
                nc.gpsimd.memset(bd8[mb], 0.0)
                for i in range(4):
                    nc.vector.tensor_copy(
                        bd8[mb][32 * i: 32 * (i + 1), bass.ds(32 * i, 32)],
                        PtT[32 * i: 32 * (i + 1), :],
                    )

            # ================= W2T build (reuses acc psum banks) =======
            # W1 = (attn*32) @ V9 into the dead gram psums: acc1 holds
            # cols 0:512, acc2[:, 192:256] the 64-tail
            w1_sb = smallp.tile([128, 2 * 576], E4, tag="w1sb", name="w1sb")
            for mb in range(2):
                vsl = v9_sb[:, 576 * mb: 576 * mb + 576]
                nc.tensor.matmul(acc1, lhsT=bd8[mb],
                                 rhs=vsl[:, 0:512], start=True, stop=True,
                                 skip_group_check=True)
                nc.tensor.matmul(acc2[:, 192:256], lhsT=bd8[mb],
                                 rhs=vsl[:, 512:576], start=True, stop=True,
                                 skip_group_check=True)
                nc.scalar.copy(w1_sb[:, 576 * mb: 576 * mb + 512], acc1)
                nc.vector.tensor_copy(
                    w1_sb[:, 576 * mb + 512: 576 * mb + 576],
                    acc2[:, 192:256])
            w13 = w1_sb[:].rearrange("p (two k) -> p two k", two=2)
            wp3 = wp_sb[:].rearrange("p (two n) -> p two n", two=2)
            for j in range(5):
                kw = 128 if j < 4 else 64
                wbuf = (acc2 if j % 2 == 0 else acc1)[0:kw, 0:256]
                nc.tensor.matmul(
                    wbuf,
                    lhsT=w13[:, :, 128 * j: 128 * j + kw],
                    rhs=wp3, start=True, stop=True, perf_mode=DR,
                    skip_group_check=True)
                dstt = w2t_t[j // 2] if j < 4 else w2t_t[2]
                dsts = (j % 2) if j < 4 else 1
                nc.scalar.activation(
                    dstt[0:kw, dsts * 256: (dsts + 1) * 256], wbuf,
                    mybir.ActivationFunctionType.Copy,
                    bias=0.0, scale=float(W2_COPY_SCALE))

            # ================= phase 2: fus + W2T@z9 =================
            w2t3 = [t[:].rearrange("p (s n) -> p s n", n=256) for t in w2t_t]
            od3 = od[:].rearrange("(two p) c -> p two c", two=2)

            def p2_attn_out(g, zap, pos):
                n0 = 512 * g
                cc = g % 4
                o2 = p2p.tile([128, 1024], BF16, tag="o2", name="o2")
                for mb in range(2):
                    po = pos[mb]
                    for r in range(4):
                        base = (4 * cc + r) * PW
                        pairs = [
                            (base, P0D, 0),
                            (base + 2 * PW, P1D, 1),
                            (ADB + base + 2, P2D, 2),
                        ]
                        for j, (X, D_, wk_) in enumerate(pairs):
                            nc.tensor.matmul(
                                po[:, 128 * r: 128 * r + 128],
                                lhsT=w2t3[wk_][:, :,
                                               128 * mb: 128 * mb + 128],
                                rhs=_pair_ap(zap, X, D_, 128),
                                start=False,
                                stop=(r == 3 and j == 2),
                                perf_mode=DR,
                                skip_group_check=True)
                    if mb == 0:
                        nc.scalar.copy(o2[:, 0:512], po)
                    else:
                        nc.vector.tensor_copy(o2[:, 512:1024], po)
                # one DMA for both halves: dst rows (p, p+128), cols n0..+512
                dst = od3[:, :, bass.ds(n0, 512)]
                src = o2[:].rearrange("p (two c) -> p two c", two=2)
                nc.sync.dma_start(out=dst, in_=src)

            for bz in range(NB):
                zP = zP0 if bz == 0 else build_pads(
                    pads, pad_d["zc"], "z", bz, E5)
                zap = zP[:]
                for cc in range(4):
                    g = 4 * bz + cc
                    pos = prefill.pop(g, None) or p2_fus(g)
                    p2_attn_out(g, zap, pos)

    _split_excess_waits(nc)
    return nc


def _prep_weights(inputs):
    wq = _merge_w(np.asarray(inputs["Wq"], np.float32),
                  np.asarray(inputs["Wq_dw"], np.float32), QK_SCALE)
    wk = _merge_w(np.asarray(inputs["Wk"], np.float32),
                  np.asarray(inputs["Wk_dw"], np.float32), QK_SCALE)
    v9 = _merge_v9(np.asarray(inputs["Wv"], np.float32),
                   np.asarray(inputs["Wv_dw"], np.float32), V9_SCALE)

    wproj = np.asarray(inputs["Wproj"], np.float32)[:, :, 0, 0]  # [256,256]
    # WprojN [c, o] mb tiles side by side: [128, 512]
    wprojN = np.zeros((128, 512), np.float32)
    wprojN[:, 0:256] = wproj[:, 0:128].T * PR_SCALE
    wprojN[:, 256:512] = wproj[:, 128:256].T * PR_SCALE

    wfus = np.asarray(inputs["Wfus"], np.float32)[:, :, 0, 0]  # [256, 192]
    wfusT = np.zeros((128, 512), np.float32)
    wfusT[:, 0:256] = wfus[:, 0:128].T          # x,y rows
    wfusT[0:64, 256:512] = wfus[:, 128:192].T   # z rows

    temp = np.asarray(inputs["temperature"], np.float32).reshape(HEADS)
    tfull = np.repeat(temp, 32).astype(np.float32)
    temp_cols = [tfull[0:128].reshape(128, 1), tfull[128:256].reshape(128, 1)]
    return wq, wk, v9, wprojN, wfusT, temp_cols


def _canvas(img, np8):
    """img [64, 128, 128] fp32 -> [128, 2*130*PW] canvas in np8: cols
    [0, LC) = AB (parts 0:64 = A padded image at pitch PW, 64:128 =
    B = A<<1col), cols [LC, 2LC) = AD (A | D = A<<1row)."""
    LC = 130 * PW
    A = np.zeros((64, 130, PW), np.float32)
    A[:, 1:129, 1:129] = img
    Af = A.reshape(64, LC)
    ext = np.zeros((64, LC + PW + 8), np.float32)
    ext[:, :LC] = Af
    out = np.zeros((128, 2 * LC), np.float32)
    out[0:64, 0:LC] = Af
    out[64:128, 0:LC] = ext[:, 1: LC + 1]
    out[0:64, LC:] = Af
    out[64:128, LC:] = ext[:, PW: LC + PW]
    return out.astype(np8)


def kernel(**inputs):
    x = np.asarray(inputs["x"], np.float32)
    y = np.asarray(inputs["y"], np.float32)
    z = np.asarray(inputs["z"], np.float32)
    B = x.shape[0]
    assert B == 8

    nc = _build_nc(*_prep_weights(inputs))

    in_maps = []
    for i in range(B):
        xi = x[i].reshape(C, N)
        yi = y[i].reshape(C, N)
        zi = z[i].reshape(C, N)
        in_maps.append({
            "xy": _bf(np.concatenate([xi, yi], axis=0)),
            "z": _bf(zi),
            "xc": _canvas(x[i], NP_E4),
            "yc": _canvas(y[i], NP_E4),
            "zc": _canvas(z[i] * Z8_SCALE, NP_E5),
        })
    res = run_bass_kernel_spmd(nc, in_maps, list(range(8)))
    out = np.stack(
        [np.asarray(res.results[i]["out"]).astype(np.float32).reshape(DIM, H, W)
         for i in range(B)]
    )
    return out


# revision 35
# speedup vs baseline: 1.0424x; 1.0424x over previous
"""Trainium2 Bass kernel for nn_LHFA_76278619177511.

Fused transposed-attention block (LHFA):
  q = dwconv3(conv1x1(x, Wq), Wq_dw)   (k from y, v from z)
  attn = softmax(l2norm(q) @ l2norm(k)^T * temp)   per-head [32,32]
  out = Wproj @ (attn @ v) + Wfus @ [x;y;z]

V2 strategy (pure DP over batch B=8 on 8 cores), key ideas:
  - fp8 DoubleRow matmuls (0.5 cycles/row, 2 K-tiles per instruction):
    the merged 1x1+dw conv (K=576) runs in 3 DR matmuls instead of 5
    bf16 passes. Pad row pitch = 144 so every DR pair stride/offset is
    16-aligned (dual-fp8 Ldweights ISA restriction).
  - q/k path entirely in fp8-e4m3 with weights pre-scaled x128; the
    scale cancels exactly in the L2 normalization.
  - The whole v path is folded into weights: W2T = (Wproj@attn@V9)^T is
    built on-device with tiny matmuls after softmax, then the attention
    output term becomes a single fused conv W2T^T @ z9 in fp8-e5m2
    accumulated INTO THE SAME PSUM as the bf16 fus conv (scales cancel:
    W2T stored x2^8, z fed x2^-8). No v slab, no attn@v, no proj.
  - Grams/sq-norms via DR on row pairs (64 pairs over the image).
  - attn output term is ~500x smaller than the fus term, so fp8 noise
    in it is invisible at the 2e-2 gate; fus stays bf16.
"""

import numpy as np
import ml_dtypes

import bass_rust
import concourse.bass as bass
import concourse.mybir as mybir
from concourse import tile as tile_mod
from concourse.tile import TileContext
from concourse.vector_clock import ScopedClock
from concourse.bass_utils import run_bass_kernel_spmd

BF16 = mybir.dt.bfloat16
F32 = mybir.dt.float32
E4 = mybir.dt.float8e4
E5 = mybir.dt.float8e5
NP_E4 = ml_dtypes.float8_e4m3
NP_E5 = ml_dtypes.float8_e5m2
NP_BF = ml_dtypes.bfloat16
DR = mybir.MatmulPerfMode.DoubleRow

C = 64          # input channels
DIM = 256       # q/k/v channels
HEADS = 8
H = W = 128
N = H * W       # 16384
PW = 144        # padded row pitch (16-aligned; image cols at [1,129))
HB = 16         # band height
NB = H // HB    # 8 bands
NRT = HB + 2    # rows per band tile
TWA = NRT * PW  # 2592: region width (AB / AD each)
ADB = 2606      # AD region base col (== 14 mod 16 so P1 stride is 16-mult)
FA = 7168       # pad tile alloc width (slice-bound slack)

# DR pair definitions: (X0(base), delta). base = hl*PW.
#   P0: (S0 @ AB+base,        S1 @ AB+base+PW)        taps rows -1, 0
#   P1: (S2 @ AB+base+2PW,    S3 @ AD+base+2)         taps row +1, col +1
#   P2: (dummy @ AD+base+2,   S4 @ AD+base+2PW+2)     tap (1,1) (A only)
P0D = PW
P1D = ADB + 2 - 2 * PW   # 2320
P2D = 2 * PW             # 288
assert P1D % 16 == 0 and ADB % 16 == 14
# weight slots [6 x 256]: 0=S0 1=S1 2=S2 3=S3 4=ZERO 5=S4
# taps (dy,dx) per slot/half for weight merging (half 0 = A rows,
# half 1 = B rows (slots 0-2, B = A<<1col) or D rows (slot 3, D = A<<1row))
SLOT_TAPS = [
    [(-1, -1), (-1, 0)],
    [(0, -1), (0, 0)],
    [(1, -1), (1, 0)],
    [(-1, 1), (0, 1)],
    [],
    [(1, 1)],
]

QK_SCALE = 128.0      # q/k conv weight prescale (cancels in l2norm)
V9_SCALE = 2.0 ** 12  # V9 prescale into e4m3
PR_SCALE = 2.0 ** 6   # WprojN prescale into e4m3
AT_SCALE = 32.0       # attn (Pt) prescale into e4m3
W2_OUT_SCALE = 2.0 ** 8   # W2T stored scale (z fed at 2^-8)
Z8_SCALE = 2.0 ** -8
# W2 psum carries AT*V9*PR = 2^23; store at 2^8 -> copy scale 2^-15
W2_COPY_SCALE = W2_OUT_SCALE / (AT_SCALE * V9_SCALE * PR_SCALE)

_PATCHED = False


def _patch_tile_drain():
    """This walrus build rejects >1 sem wait on a CTRL (Drain) instruction;
    split the TileContext tail-drain waits onto individual nops."""
    global _PATCHED
    if _PATCHED:
        return
    _PATCHED = True

    def _drain_and_barrier(self, tick_clock, wait_clock):
        nc = self.nc
        drain_inst = nc.sync.drain()
        wait_clock.add_sem_waits(
            drain_inst.ins, ScopedClock({None: tick_clock.global_clock})
        )
        si = drain_inst.ins.sync_info
        waits = list(si.on_wait or [])
        if len(waits) > 1:
            si.on_wait = waits[:1]
            for w in waits[1:]:
                nop = nc.sync.nop(nofuse=True, hint="split_wait")
                nop.ins.sync_info = bass_rust.SyncInfo(on_wait=[w], on_update=[])
        nc.all_engine_barrier()
        assert self.sems is not None
        popped = nc._tile_sem_poison_stack.pop()
        assert popped is self._sem_poison
        nc.clear_and_free_semaphores(list(self.sems.allocated().values()))
        nc.all_engine_barrier()

    tile_mod.TileContext._drain_and_barrier = _drain_and_barrier
    try:
        from concourse import tile_utils
        tile_utils.max_sbuf_usage = 208 * 1024
    except Exception:
        pass


def _split_excess_waits(nc, max_waits=1):
    """This walrus build caps sem waits per instruction encoding; hoist
    excess waits onto preceding same-engine NoOps (queues are in-order,
    so a wait satisfied on an earlier instruction orders the later one)."""
    import bass_rust as _br

    ctr = [0]
    for f in nc.m.functions:
        for blk in f.blocks:
            out = []
            for inst in blk.instructions:
                si = inst.sync_info
                waits = list(si.on_wait) if (si and si.on_wait) else []
                if len(waits) > max_waits:
                    keep = waits[:max_waits]
                    extra = waits[max_waits:]
                    si.on_wait = keep
                    for w in extra:
                        ctr[0] += 1
                        nop = _br.InstNoOp(name=f"wsplit-{ctr[0]}", ins=[], outs=[])
                        nop.engine = inst.engine
                        nop.sync_info = _br.SyncInfo(on_wait=[w], on_update=[])
                        try:
                            nc.register_instruction(nop, overwrite=True)
                        except Exception:
                            pass
                        out.append(nop)
                out.append(inst)
            blk.instructions[:] = out


def _merge_w(W1, Wdw, scale):
    """-> [128, 6, 256] float32 merged conv weights in slot layout.
    [p=(half,chan), slot, outch] = dw[out, tap(slot, half)] * W1[out, chan]."""
    out = np.zeros((128, 6, 256), np.float32)
    W1 = W1[:, :, 0, 0]  # [256, 64]
    for s, taps in enumerate(SLOT_TAPS):
        for half, (dy, dx) in enumerate(taps):
            hh = half if s != 5 else 0
            out[hh * 64: (hh + 1) * 64, s, :] = (
                Wdw[:, 0, 1 + dy, 1 + dx][:, None] * W1
            ).T * scale
    return out


def _merge_v9(W1, Wdw, scale):
    """-> [256, 576] float32: V9[d, s*128 + half*64 + c] for slots 0-3 +
    slot5(A half) packed as k-chunk order s in 0..4 (chunk4 = slot5 taps)."""
    out = np.zeros((256, 640), np.float32)
    W1 = W1[:, :, 0, 0]  # [256, 64]
    for s, taps in enumerate(SLOT_TAPS):
        if s == 4:
            continue
        kc = s if s < 4 else 4
        for half, (dy, dx) in enumerate(taps):
            hh = half if s != 5 else 0
            out[:, kc * 128 + hh * 64: kc * 128 + (hh + 1) * 64] = (
                Wdw[:, 0, 1 + dy, 1 + dx][:, None] * W1
            ) * scale
    return out[:, :576]


def _bf(a):
    return np.ascontiguousarray(a).astype(NP_BF)


def _pair_ap(tile_ap, X, delta, width):
    """[128, 2, width] AP: members at cols X and X+delta."""
    return tile_ap[:, X: X + 2 * delta].rearrange(
        "p (two m) -> p two m", two=2
    )[:, :, 0:width]


def _build_nc(wq, wk, v9, wprojN, wfusT, temp_cols):
    """Build the Bass module. Weight arrays pre-merged/scaled fp32."""
    _patch_tile_drain()
    nc = bass.Bass()

    # xy interleaved for the fus loads: parts 0:64 = x, 64:128 = y
    xyd = nc.declare_dram_parameter("xy", [128, N], BF16, isOutput=False)
    zd = nc.declare_dram_parameter("z", [C, N], BF16, isOutput=False)
    # pre-padded, pre-shifted canvases [128, 2*LC]: cols 0:LC = AB
    # (parts 0:64 = A padded image, 64:128 = B = A<<1col), cols LC:2LC =
    # AD (A | D = A<<1row)
    LC = 130 * PW
    pad_d = {}
    for nm, dt8 in (("xc", E4), ("yc", E4), ("zc", E5)):
        pad_d[nm] = nc.declare_dram_parameter(
            nm, [128, 2 * LC], dt8, isOutput=False)
    od = nc.declare_dram_parameter("out", [DIM, N], BF16, isOutput=True)

    wq_d = nc.inline_tensor(
        np.ascontiguousarray(wq.reshape(128, 6 * 256)).astype(NP_E4), name="wq9")
    wk_d = nc.inline_tensor(
        np.ascontiguousarray(wk.reshape(128, 6 * 256)).astype(NP_E4), name="wk9")
    # V9 [256,576] -> 2 mb tiles side by side [128, 1152]
    v9_2 = np.concatenate([v9[0:128], v9[128:256]], axis=1)
    v9_d = nc.inline_tensor(np.ascontiguousarray(v9_2).astype(NP_E4), name="v9")
    wp_d = nc.inline_tensor(
        np.ascontiguousarray(wprojN).astype(NP_E4), name="wprojN")  # [128, 512]
    wf_d = nc.inline_tensor(_bf(wfusT), name="wfusT")               # [128, 512]
    tc0_d = nc.inline_tensor(np.ascontiguousarray(temp_cols[0]), name="tcol0")
    tc1_d = nc.inline_tensor(np.ascontiguousarray(temp_cols[1]), name="tcol1")
    id_d = nc.inline_tensor(np.eye(128, dtype=NP_BF), name="ident")

    with TileContext(nc) as tc:
        import contextlib

        with contextlib.ExitStack() as ctx:
            wpool = ctx.enter_context(tc.tile_pool(name="wpool", bufs=1))
            pads = ctx.enter_context(tc.tile_pool(name="pads", bufs=3))
            qkp = ctx.enter_context(tc.tile_pool(name="qkp", bufs=3))
            smallp = ctx.enter_context(tc.tile_pool(name="smallp", bufs=2))

            # --- weights to SBUF ---
            wq_sb = wpool.tile([128, 6 * 256], E4, tag="wq")
            wk_sb = wpool.tile([128, 6 * 256], E4, tag="wk")
            v9_sb = wpool.tile([128, 2 * 576], E4, tag="v9")
            wp_sb = wpool.tile([128, 512], E4, tag="wp")
            wf_sb = wpool.tile([128, 512], BF16, tag="wf")
            # w2t as 3 separate pair tiles so phase-2 DRs only wait on the
            # slots they read
            w2t_t = [wpool.tile([128, 512], E5, tag=f"w2t{k}",
                                name=f"w2t{k}") for k in range(3)]
            ident_sb = wpool.tile([128, 128], BF16, tag="ident")
            tcol = [wpool.tile([128, 1], F32, tag=f"tc{i}", name=f"tcol{i}")
                    for i in range(2)]

            # --- pass-1 psums (qk pool created last, closes first: LIFO) ---
            p1stack = ctx.enter_context(contextlib.ExitStack())
            qkstack = ctx.enter_context(contextlib.ExitStack())
            ps_acc = p1stack.enter_context(
                tc.tile_pool(name="ps_acc", bufs=1, space="PSUM"))
            ps_qk = qkstack.enter_context(
                tc.tile_pool(name="ps_qk", bufs=3, space="PSUM"))
            acc1 = ps_acc.tile([128, 512], F32, tag="acc1")
            acc2 = ps_acc.tile([128, 256], F32, tag="acc2")
            par_all = acc1[:, 0:256]
            pgq = acc1[:, 256:512]
            pgk = acc2

            def build_pads(pool, dc, name, band, dt8):
                """Load the [128, FA] padded AB|AD tile for one band in ONE
                two-range DMA from the host-baked shifted canvas."""
                P = pool.tile([128, FA], dt8, tag=f"pad{name}")
                o = (HB * band) * PW
                src = dc[:].rearrange(
                    "p (two l) -> p two l", l=LC)[:, :, o: o + TWA]
                dst = P[:][:, 0: 2 * ADB].rearrange(
                    "p (two w) -> p two w", two=2)[:, :, 0:TWA]
                nc.sync.dma_start(out=dst, in_=src)
                return P

            # band-0 pads first so the first conv's data leads the DMA
            # device queue; weights interleave behind them
            xP0 = build_pads(pads, pad_d["xc"], "x", 0, E4)
            nc.scalar.dma_start(out=wq_sb, in_=wq_d[:])
            yP0 = build_pads(pads, pad_d["yc"], "y", 0, E4)
            nc.scalar.dma_start(out=wk_sb, in_=wk_d[:])
            nc.gpsimd.dma_start(out=v9_sb, in_=v9_d[:])
            nc.gpsimd.dma_start(out=wp_sb, in_=wp_d[:])
            nc.gpsimd.dma_start(out=wf_sb, in_=wf_d[:])
            nc.gpsimd.dma_start(out=tcol[0], in_=tc0_d[:])
            nc.gpsimd.dma_start(out=tcol[1], in_=tc1_d[:])
            nc.gpsimd.dma_start(out=ident_sb, in_=id_d[:])

            def conv_drs(P, w_sb, pt, base, start_tag):
                """3 DR matmuls accumulating one row's conv into pt [128,256]."""
                ap = P[:]
                w3 = w_sb[:].rearrange("p (s n) -> p s n", n=256)
                pairs = [
                    (base, P0D, 0),
                    (base + 2 * PW, P1D, 2),
                    (ADB + base + 2, P2D, 4),
                ]
                for j, (X, D_, ws) in enumerate(pairs):
                    nc.tensor.matmul(
                        pt,
                        lhsT=_pair_ap(ap, X, D_, 128),
                        rhs=w3[:, ws: ws + 2, :],
                        start=(j == 0),
                        stop=(j == 2),
                        perf_mode=DR,
                    )

            # ================= pass 1: q/k convs + grams =================
            pend_gram = None  # (cat tile, first, last)

            def emit_grams(cat, first, last):
                cat3 = cat[:].rearrange("p (two c) -> p two c", two=2)
                for mb in range(2):
                    qsl = cat3[:, :, 256 + 128 * mb: 256 + 128 * mb + 128]
                    ksl = cat3[:, :, 128 * mb: 128 * mb + 128]
                    nc.tensor.matmul(
                        par_all[:, bass.ds(mb * 128, 128)],
                        lhsT=qsl, rhs=ksl, start=first, stop=last,
                        perf_mode=DR, skip_group_check=True)
                    nc.tensor.matmul(
                        pgq[:, bass.ds(mb * 128, 128)],
                        lhsT=qsl, rhs=qsl, start=first, stop=last,
                        perf_mode=DR, skip_group_check=True)
                    nc.tensor.matmul(
                        pgk[:, bass.ds(mb * 128, 128)],
                        lhsT=ksl, rhs=ksl, start=first, stop=last,
                        perf_mode=DR, skip_group_check=True)

            # persistent fus-input slabs, preloaded during pass 1 (the DMA
            # device has slack there; phase 2 then only moves pads + output)
            fuspool = ctx.enter_context(tc.tile_pool(name="fusp", bufs=1))
            xy_slab = fuspool.tile([128, N], BF16, tag="xyslab", name="xyslab")
            z_slab = fuspool.tile([64, N], BF16, tag="zslab", name="zslab")

            for b in range(NB):
                xP = xP0 if b == 0 else build_pads(
                    pads, pad_d["xc"], "x", b, E4)
                yP = yP0 if b == 0 else build_pads(
                    pads, pad_d["yc"], "y", b, E4)
                n0 = b * (N // NB)
                nc.gpsimd.dma_start(
                    out=xy_slab[:, bass.ds(n0, N // NB)],
                    in_=xyd[:, bass.ds(n0, N // NB)])
                nc.gpsimd.dma_start(
                    out=z_slab[:, bass.ds(n0, N // NB)],
                    in_=zd[:, bass.ds(n0, N // NB)])
                for hl2 in range(HB // 2):
                    pqk = ps_qk.tile([128, 1024], F32, tag="pqk")
                    for half in range(2):  # even/odd row of the pair
                        base = (2 * hl2 + half) * PW
                        o = 512 * half
                        conv_drs(yP, wk_sb, pqk[:, o: o + 256], base, "k")
                        conv_drs(xP, wq_sb, pqk[:, o + 256: o + 512], base, "q")
                    cat = qkp.tile([128, 1024], E4, tag="cat")
                    nc.scalar.copy(cat[:, 0:512], pqk[:, 0:512])
                    nc.vector.tensor_copy(cat[:, 512:1024], pqk[:, 512:1024])
                    if pend_gram is not None:
                        emit_grams(*pend_gram)
                    pg = 8 * b + hl2
                    pend_gram = (cat, pg == 0, pg == 63)
            emit_grams(*pend_gram)
            qkstack.close()
            # prefetch z band-0 pads early (DMA only; overlaps pass-1 tail)
            zP0 = build_pads(pads, pad_d["zc"], "z", 0, E5)

            # --- phase-2 pools + prefill: fus matmuls run on the PE while
            # the softmax chain occupies ACT/DVE ---
            p2p = ctx.enter_context(tc.tile_pool(name="p2p", bufs=4))
            ps_o = ctx.enter_context(
                tc.tile_pool(name="ps_o", bufs=6, space="PSUM"))

            def p2_fus(g):
                n0 = 512 * g
                pos = []
                for mb in range(2):
                    po = ps_o.tile([128, 512], F32, tag="po")
                    nc.tensor.matmul(
                        po, lhsT=wf_sb[:, bass.ds(mb * 128, 128)],
                        rhs=xy_slab[:, bass.ds(n0, 512)],
                        start=True, stop=False)
                    nc.tensor.matmul(
                        po, lhsT=wf_sb[0:64, bass.ds(256 + mb * 128, 128)],
                        rhs=z_slab[:, bass.ds(n0, 512)],
                        start=False, stop=False)
                    pos.append(po)
                return pos

            # slot 4 (pair tile 2, first half) is zeros; k-chunk 4 -> slot 5
            nc.gpsimd.memset(w2t_t[2][:, 0:256], 0.0)
            nc.gpsimd.memset(w2t_t[2][64:128, 256:512], 0.0)
            prefill = {g: p2_fus(g) for g in range(3)}

            # ================= softmax on per-head [32,32] =================
            ar_sb = [smallp.tile([128, 128], F32, tag=f"arsb{mb}",
                                 name=f"arsb{mb}") for mb in range(2)]
            nc.scalar.copy(ar_sb[0], par_all[:, 0:128])
            nc.scalar.copy(ar_sb[1], par_all[:, 128:256])
            bd8 = [smallp.tile([128, 128], E4, tag=f"bd{mb}",
                               name=f"bdiag{mb}") for mb in range(2)]
            for mb in range(2):
                scr = smallp.tile([128, 128], F32, tag="scr")
                rnq_c = smallp.tile([128, 1], F32, tag="rnq")
                rnk_c = smallp.tile([128, 1], F32, tag="rnk")
                for g_ps, dst in ((pgq, rnq_c), (pgk, rnk_c)):
                    ssum = smallp.tile([128, 1], F32, tag="ssum")
                    nc.vector.tensor_mul(
                        scr, g_ps[:, bass.ds(mb * 128, 128)], ident_sb)
                    nc.vector.reduce_sum(
                        out=ssum, in_=scr, axis=mybir.AxisListType.X)
                    nc.scalar.sqrt(ssum, ssum)
                    nc.vector.tensor_scalar_max(ssum, ssum, 1e-12)
                    nc.vector.reciprocal(dst, ssum)
                rnqt = smallp.tile([128, 1], F32, tag="rnqt")
                nc.vector.tensor_mul(rnqt, rnq_c, tcol[mb])

                hd = smallp.tile([128, 32], F32, tag="hd")
                for i in range(4):
                    nc.vector.tensor_copy(
                        hd[32 * i: 32 * (i + 1), :],
                        ar_sb[mb][32 * i: 32 * (i + 1), bass.ds(32 * i, 32)],
                    )
                hds = smallp.tile([128, 32], F32, tag="hds")
                nc.scalar.activation(
                    hds, hd, mybir.ActivationFunctionType.Copy,
                    bias=0.0, scale=rnqt)
                hdT = smallp.tile([128, 32], F32, tag="hdT")
                nc.vector.transpose(hdT, hds)
                hdTs = smallp.tile([128, 32], F32, tag="hdTs")
                nc.scalar.activation(
                    hdTs, hdT, mybir.ActivationFunctionType.Copy,
                    bias=0.0, scale=rnk_c)
                hd3 = smallp.tile([128, 32], F32, tag="hd3")
                nc.vector.transpose(hd3, hdTs)
                nmx = smallp.tile([128, 1], F32, tag="nmx")
                nc.vector.reduce_max(
                    out=nmx, in_=hd3, axis=mybir.AxisListType.X, negate=True)
                ex = smallp.tile([128, 32], F32, tag="ex")
                nc.scalar.activation(
                    ex, hd3, mybir.ActivationFunctionType.Exp,
                    bias=nmx, scale=1.0)
                sm = smallp.tile([128, 1], F32, tag="sm")
                nc.vector.reduce_sum(out=sm, in_=ex, axis=mybir.AxisListType.X)
                rsm = smallp.tile([128, 1], F32, tag="rsm")
                nc.vector.reciprocal(rsm, sm)
                rsm32 = smallp.tile([128, 1], F32, tag="rsm32")
                nc.scalar.activation(
                    rsm32, rsm, mybir.ActivationFunctionType.Copy,
                    bias=0.0, scale=AT_SCALE)
                Pt = smallp.tile([128, 32], F32, tag="Pt")
                nc.scalar.activation(
                    Pt, ex, mybir.ActivationFunctionType.Copy,
                    bias=0.0, scale=rsm32)
                PtT = smallp.tile([128, 32], F32, tag="PtT")
                nc.vector.transpose(PtT, Pt)
                nc.gpsimd.memset(bd8[mb], 0.0)
                for i in range(4):
                    nc.vector.tensor_copy(
                        bd8[mb][32 * i: 32 * (i + 1), bass.ds(32 * i, 32)],
                        PtT[32 * i: 32 * (i + 1), :],
                    )

            # ================= W2T build (reuses acc psum banks) =======
            # W1 = (attn*32) @ V9 into the dead gram psums: acc1 holds
            # cols 0:512, acc2[:, 192:256] the 64-tail
            w1_sb = smallp.tile([128, 2 * 576], E4, tag="w1sb", name="w1sb")
            for mb in range(2):
                vsl = v9_sb[:, 576 * mb: 576 * mb + 576]
                nc.tensor.matmul(acc1, lhsT=bd8[mb],
                                 rhs=vsl[:, 0:512], start=True, stop=True,
                                 skip_group_check=True)
                nc.tensor.matmul(acc2[:, 192:256], lhsT=bd8[mb],
                                 rhs=vsl[:, 512:576], start=True, stop=True,
                                 skip_group_check=True)
                nc.scalar.copy(w1_sb[:, 576 * mb: 576 * mb + 512], acc1)
                nc.vector.tensor_copy(
                    w1_sb[:, 576 * mb + 512: 576 * mb + 576],
                    acc2[:, 192:256])
            w13 = w1_sb[:].rearrange("p (two k) -> p two k", two=2)
            wp3 = wp_sb[:].rearrange("p (two n) -> p two n", two=2)
            for j in range(5):
                kw = 128 if j < 4 else 64
                wbuf = (acc2 if j % 2 == 0 else acc1)[0:kw, 0:256]
                nc.tensor.matmul(
                    wbuf,
                    lhsT=w13[:, :, 128 * j: 128 * j + kw],
                    rhs=wp3, start=True, stop=True, perf_mode=DR,
                    skip_group_check=True)
                dstt = w2t_t[j // 2] if j < 4 else w2t_t[2]
                dsts = (j % 2) if j < 4 else 1
                nc.scalar.activation(
                    dstt[0:kw, dsts * 256: (dsts + 1) * 256], wbuf,
                    mybir.ActivationFunctionType.Copy,
                    bias=0.0, scale=float(W2_COPY_SCALE))

            # ================= phase 2: fus + W2T@z9 =================
            w2t3 = [t[:].rearrange("p (s n) -> p s n", n=256) for t in w2t_t]
            od3 = od[:].rearrange("(two p) c -> p two c", two=2)

            def p2_attn_out(g, zap, pos):
                n0 = 512 * g
                cc = g % 4
                o2 = p2p.tile([128, 1024], BF16, tag="o2", name="o2")
                for mb in range(2):
                    po = pos[mb]
                    for r in range(4):
                        base = (4 * cc + r) * PW
                        pairs = [
                            (base, P0D, 0),
                            (base + 2 * PW, P1D, 1),
                            (ADB + base + 2, P2D, 2),
                        ]
                        for j, (X, D_, wk_) in enumerate(pairs):
                            nc.tensor.matmul(
                                po[:, 128 * r: 128 * r + 128],
                                lhsT=w2t3[wk_][:, :,
                                               128 * mb: 128 * mb + 128],
                                rhs=_pair_ap(zap, X, D_, 128),
                                start=False,
                                stop=(r == 3 and j == 2),
                                perf_mode=DR,
                                skip_group_check=True)
                    if mb == 0:
                        nc.scalar.copy(o2[:, 0:512], po)
                    else:
                        nc.vector.tensor_copy(o2[:, 512:1024], po)
                # one DMA for both halves: dst rows (p, p+128), cols n0..+512
                dst = od3[:, :, bass.ds(n0, 512)]
                src = o2[:].rearrange("p (two c) -> p two c", two=2)
                nc.sync.dma_start(out=dst, in_=src)

            for bz in range(NB):
                zP = zP0 if bz == 0 else build_pads(
                    pads, pad_d["zc"], "z", bz, E5)
                zap = zP[:]
                for cc in range(4):
                    g = 4 * bz + cc
                    pos = prefill.pop(g, None) or p2_fus(g)
                    p2_attn_out(g, zap, pos)

    _split_excess_waits(nc)
    return nc


def _prep_weights(inputs):
    wq = _merge_w(np.asarray(inputs["Wq"], np.float32),
                  np.asarray(inputs["Wq_dw"], np.float32), QK_SCALE)
    wk = _merge_w(np.asarray(inputs["Wk"], np.float32),
                  np.asarray(inputs["Wk_dw"], np.float32), QK_SCALE)
    v9 = _merge_v9(np.asarray(inputs["Wv"], np.float32),
                   np.asarray(inputs["Wv_dw"], np.float32), V9_SCALE)

    wproj = np.asarray(inputs["Wproj"], np.float32)[:, :, 0, 0]  # [256,256]
    # WprojN [c, o] mb tiles side by side: [128, 512]
    wprojN = np.zeros((128, 512), np.float32)
    wprojN[:, 0:256] = wproj[:, 0:128].T * PR_SCALE
    wprojN[:, 256:512] = wproj[:, 128:256].T * PR_SCALE

    wfus = np.asarray(inputs["Wfus"], np.float32)[:, :, 0, 0]  # [256, 192]
    wfusT = np.zeros((128, 512), np.float32)
    wfusT[:, 0:256] = wfus[:, 0:128].T          # x,y rows
    wfusT[0:64, 256:512] = wfus[:, 128:192].T   # z rows

    temp = np.asarray(inputs["temperature"], np.float32).reshape(HEADS)
    tfull = np.repeat(temp, 32).astype(np.float32)
    temp_cols = [tfull[0:128].reshape(128, 1), tfull[128:256].reshape(128, 1)]
    return wq, wk, v9, wprojN, wfusT, temp_cols


def _canvas(img, np8):
    """img [64, 128, 128] fp32 -> [128, 2*130*PW] canvas in np8: cols
    [0, LC) = AB (parts 0:64 = A padded image at pitch PW, 64:128 =
    B = A<<1col), cols [LC, 2LC) = AD (A | D = A<<1row)."""
    LC = 130 * PW
    A = np.zeros((64, 130, PW), np.float32)
    A[:, 1:129, 1:129] = img
    Af = A.reshape(64, LC)
    ext = np.zeros((64, LC + PW + 8), np.float32)
    ext[:, :LC] = Af
    out = np.zeros((128, 2 * LC), np.float32)
    out[0:64, 0:LC] = Af
    out[64:128, 0:LC] = ext[:, 1: LC + 1]
    out[0:64, LC:] = Af
    out[64:128, LC:] = ext[:, PW: LC + PW]
    return out.astype(np8)


def kernel(**inputs):
    x = np.asarray(inputs["x"], np.float32)
    y = np.asarray(inputs["y"], np.float32)
    z = np.asarray(inputs["z"], np.float32)
    B = x.shape[0]
    assert B == 8

    nc = _build_nc(*_prep_weights(inputs))

    in_maps = []
    for i in range(B):
        xi = x[i].reshape(C, N)
        yi = y[i].reshape(C, N)
        zi = z[i].reshape(C, N)
        in_maps.append({
            "xy": _bf(np.concatenate([xi, yi], axis=0)),
            "z": _bf(zi),
            "xc": _canvas(x[i], NP_E4),
            "yc": _canvas(y[i], NP_E4),
            "zc": _canvas(z[i] * Z8_SCALE, NP_E5),
        })
    res = run_bass_kernel_spmd(nc, in_maps, list(range(8)))
    out = np.stack(
        [np.asarray(res.results[i]["out"]).astype(np.float32).reshape(DIM, H, W)
         for i in range(B)]
    )
    return out


# revision 36
# speedup vs baseline: 1.0744x; 1.0308x over previous
"""Trainium2 Bass kernel for nn_LHFA_76278619177511.

Fused transposed-attention block (LHFA):
  q = dwconv3(conv1x1(x, Wq), Wq_dw)   (k from y, v from z)
  attn = softmax(l2norm(q) @ l2norm(k)^T * temp)   per-head [32,32]
  out = Wproj @ (attn @ v) + Wfus @ [x;y;z]

V2 strategy (pure DP over batch B=8 on 8 cores), key ideas:
  - fp8 DoubleRow matmuls (0.5 cycles/row, 2 K-tiles per instruction):
    the merged 1x1+dw conv (K=576) runs in 3 DR matmuls instead of 5
    bf16 passes. Pad row pitch = 144 so every DR pair stride/offset is
    16-aligned (dual-fp8 Ldweights ISA restriction).
  - q/k path entirely in fp8-e4m3 with weights pre-scaled x128; the
    scale cancels exactly in the L2 normalization.
  - The whole v path is folded into weights: W2T = (Wproj@attn@V9)^T is
    built on-device with tiny matmuls after softmax, then the attention
    output term becomes a single fused conv W2T^T @ z9 in fp8-e5m2
    accumulated INTO THE SAME PSUM as the bf16 fus conv (scales cancel:
    W2T stored x2^8, z fed x2^-8). No v slab, no attn@v, no proj.
  - Grams/sq-norms via DR on row pairs (64 pairs over the image).
  - attn output term is ~500x smaller than the fus term, so fp8 noise
    in it is invisible at the 2e-2 gate; fus stays bf16.
"""

import numpy as np
import ml_dtypes

import bass_rust
import concourse.bass as bass
import concourse.mybir as mybir
from concourse import tile as tile_mod
from concourse.tile import TileContext
from concourse.vector_clock import ScopedClock
from concourse.bass_utils import run_bass_kernel_spmd

BF16 = mybir.dt.bfloat16
F32 = mybir.dt.float32
E4 = mybir.dt.float8e4
E5 = mybir.dt.float8e5
NP_E4 = ml_dtypes.float8_e4m3
NP_E5 = ml_dtypes.float8_e5m2
NP_BF = ml_dtypes.bfloat16
DR = mybir.MatmulPerfMode.DoubleRow

C = 64          # input channels
DIM = 256       # q/k/v channels
HEADS = 8
H = W = 128
N = H * W       # 16384
PW = 144        # padded row pitch (16-aligned; image cols at [1,129))
HB = 16         # band height
NB = H // HB    # 8 bands
NRT = HB + 2    # rows per band tile
TWA = NRT * PW  # 2592: region width (AB / AD each)
ADB = 2606      # AD region base col (== 14 mod 16 so P1 stride is 16-mult)
FA = 7168       # pad tile alloc width (slice-bound slack)

# DR pair definitions: (X0(base), delta). base = hl*PW.
#   P0: (S0 @ AB+base,        S1 @ AB+base+PW)        taps rows -1, 0
#   P1: (S2 @ AB+base+2PW,    S3 @ AD+base+2)         taps row +1, col +1
#   P2: (dummy @ AD+base+2,   S4 @ AD+base+2PW+2)     tap (1,1) (A only)
P0D = PW
P1D = ADB + 2 - 2 * PW   # 2320
P2D = 2 * PW             # 288
assert P1D % 16 == 0 and ADB % 16 == 14
# weight slots [6 x 256]: 0=S0 1=S1 2=S2 3=S3 4=ZERO 5=S4
# taps (dy,dx) per slot/half for weight merging (half 0 = A rows,
# half 1 = B rows (slots 0-2, B = A<<1col) or D rows (slot 3, D = A<<1row))
SLOT_TAPS = [
    [(-1, -1), (-1, 0)],
    [(0, -1), (0, 0)],
    [(1, -1), (1, 0)],
    [(-1, 1), (0, 1)],
    [],
    [(1, 1)],
]

QK_SCALE = 128.0      # q/k conv weight prescale (cancels in l2norm)
V9_SCALE = 2.0 ** 12  # V9 prescale into e4m3
PR_SCALE = 2.0 ** 6   # WprojN prescale into e4m3
AT_SCALE = 32.0       # attn (Pt) prescale into e4m3
W2_OUT_SCALE = 2.0 ** 8   # W2T stored scale (z fed at 2^-8)
Z8_SCALE = 2.0 ** -8
# W2 psum carries AT*V9*PR = 2^23; store at 2^8 -> copy scale 2^-15
W2_COPY_SCALE = W2_OUT_SCALE / (AT_SCALE * V9_SCALE * PR_SCALE)

_PATCHED = False


def _patch_tile_drain():
    """This walrus build rejects >1 sem wait on a CTRL (Drain) instruction;
    split the TileContext tail-drain waits onto individual nops."""
    global _PATCHED
    if _PATCHED:
        return
    _PATCHED = True

    def _drain_and_barrier(self, tick_clock, wait_clock):
        nc = self.nc
        drain_inst = nc.sync.drain()
        wait_clock.add_sem_waits(
            drain_inst.ins, ScopedClock({None: tick_clock.global_clock})
        )
        si = drain_inst.ins.sync_info
        waits = list(si.on_wait or [])
        if len(waits) > 1:
            si.on_wait = waits[:1]
            for w in waits[1:]:
                nop = nc.sync.nop(nofuse=True, hint="split_wait")
                nop.ins.sync_info = bass_rust.SyncInfo(on_wait=[w], on_update=[])
        nc.all_engine_barrier()
        assert self.sems is not None
        popped = nc._tile_sem_poison_stack.pop()
        assert popped is self._sem_poison
        nc.clear_and_free_semaphores(list(self.sems.allocated().values()))
        nc.all_engine_barrier()

    tile_mod.TileContext._drain_and_barrier = _drain_and_barrier
    try:
        from concourse import tile_utils
        tile_utils.max_sbuf_usage = 208 * 1024
    except Exception:
        pass


def _split_excess_waits(nc, max_waits=1):
    """This walrus build caps sem waits per instruction encoding; hoist
    excess waits onto preceding same-engine NoOps (queues are in-order,
    so a wait satisfied on an earlier instruction orders the later one)."""
    import bass_rust as _br

    ctr = [0]
    for f in nc.m.functions:
        for blk in f.blocks:
            out = []
            for inst in blk.instructions:
                si = inst.sync_info
                waits = list(si.on_wait) if (si and si.on_wait) else []
                if len(waits) > max_waits:
                    keep = waits[:max_waits]
                    extra = waits[max_waits:]
                    si.on_wait = keep
                    for w in extra:
                        ctr[0] += 1
                        nop = _br.InstNoOp(name=f"wsplit-{ctr[0]}", ins=[], outs=[])
                        nop.engine = inst.engine
                        nop.sync_info = _br.SyncInfo(on_wait=[w], on_update=[])
                        try:
                            nc.register_instruction(nop, overwrite=True)
                        except Exception:
                            pass
                        out.append(nop)
                out.append(inst)
            blk.instructions[:] = out


def _merge_w(W1, Wdw, scale):
    """-> [128, 6, 256] float32 merged conv weights in slot layout.
    [p=(half,chan), slot, outch] = dw[out, tap(slot, half)] * W1[out, chan]."""
    out = np.zeros((128, 6, 256), np.float32)
    W1 = W1[:, :, 0, 0]  # [256, 64]
    for s, taps in enumerate(SLOT_TAPS):
        for half, (dy, dx) in enumerate(taps):
            hh = half if s != 5 else 0
            out[hh * 64: (hh + 1) * 64, s, :] = (
                Wdw[:, 0, 1 + dy, 1 + dx][:, None] * W1
            ).T * scale
    return out


def _merge_v9(W1, Wdw, scale):
    """-> [256, 576] float32: V9[d, s*128 + half*64 + c] for slots 0-3 +
    slot5(A half) packed as k-chunk order s in 0..4 (chunk4 = slot5 taps)."""
    out = np.zeros((256, 640), np.float32)
    W1 = W1[:, :, 0, 0]  # [256, 64]
    for s, taps in enumerate(SLOT_TAPS):
        if s == 4:
            continue
        kc = s if s < 4 else 4
        for half, (dy, dx) in enumerate(taps):
            hh = half if s != 5 else 0
            out[:, kc * 128 + hh * 64: kc * 128 + (hh + 1) * 64] = (
                Wdw[:, 0, 1 + dy, 1 + dx][:, None] * W1
            ) * scale
    return out[:, :576]


def _bf(a):
    return np.ascontiguousarray(a).astype(NP_BF)


def _pair_ap(tile_ap, X, delta, width):
    """[128, 2, width] AP: members at cols X and X+delta."""
    return tile_ap[:, X: X + 2 * delta].rearrange(
        "p (two m) -> p two m", two=2
    )[:, :, 0:width]


def _build_nc(wq, wk, v9, wprojN, wfusT, temp_cols):
    """Build the Bass module. Weight arrays pre-merged/scaled fp32."""
    _patch_tile_drain()
    nc = bass.Bass()

    # xy interleaved for the fus loads: parts 0:64 = x, 64:128 = y
    xyd = nc.declare_dram_parameter("xy", [128, N], BF16, isOutput=False)
    zd = nc.declare_dram_parameter("z", [C, N], BF16, isOutput=False)
    # pre-padded, pre-shifted canvases [128, 2*LC]: cols 0:LC = AB
    # (parts 0:64 = A padded image, 64:128 = B = A<<1col), cols LC:2LC =
    # AD (A | D = A<<1row)
    LC = 130 * PW
    pad_d = {}
    for nm, dt8 in (("xc", E4), ("yc", E4), ("zc", E5)):
        pad_d[nm] = nc.declare_dram_parameter(
            nm, [128, 2 * LC], dt8, isOutput=False)
    od = nc.declare_dram_parameter("out", [DIM, N], BF16, isOutput=True)

    wq_d = nc.inline_tensor(
        np.ascontiguousarray(wq.reshape(128, 6 * 256)).astype(NP_E4), name="wq9")
    wk_d = nc.inline_tensor(
        np.ascontiguousarray(wk.reshape(128, 6 * 256)).astype(NP_E4), name="wk9")
    # V9 [256,576] -> 2 mb tiles side by side [128, 1152]
    v9_2 = np.concatenate([v9[0:128], v9[128:256]], axis=1)
    v9_d = nc.inline_tensor(np.ascontiguousarray(v9_2).astype(NP_E4), name="v9")
    wp_d = nc.inline_tensor(
        np.ascontiguousarray(wprojN).astype(NP_E4), name="wprojN")  # [128, 512]
    wf_d = nc.inline_tensor(_bf(wfusT), name="wfusT")               # [128, 512]
    tc0_d = nc.inline_tensor(np.ascontiguousarray(temp_cols[0]), name="tcol0")
    tc1_d = nc.inline_tensor(np.ascontiguousarray(temp_cols[1]), name="tcol1")
    id_d = nc.inline_tensor(np.eye(128, dtype=NP_BF), name="ident")

    with TileContext(nc) as tc:
        import contextlib

        with contextlib.ExitStack() as ctx:
            wpool = ctx.enter_context(tc.tile_pool(name="wpool", bufs=1))
            pads = ctx.enter_context(tc.tile_pool(name="pads", bufs=3))
            qkp = ctx.enter_context(tc.tile_pool(name="qkp", bufs=3))
            smallp = ctx.enter_context(tc.tile_pool(name="smallp", bufs=2))

            # --- weights to SBUF ---
            wq_sb = wpool.tile([128, 6 * 256], E4, tag="wq")
            wk_sb = wpool.tile([128, 6 * 256], E4, tag="wk")
            v9_sb = wpool.tile([128, 2 * 576], E4, tag="v9")
            wp_sb = wpool.tile([128, 512], E4, tag="wp")
            wf_sb = wpool.tile([128, 512], BF16, tag="wf")
            # w2t as 3 separate pair tiles so phase-2 DRs only wait on the
            # slots they read
            w2t_t = [wpool.tile([128, 512], E5, tag=f"w2t{k}",
                                name=f"w2t{k}") for k in range(3)]
            ident_sb = wpool.tile([128, 128], BF16, tag="ident")
            tcol = [wpool.tile([128, 1], F32, tag=f"tc{i}", name=f"tcol{i}")
                    for i in range(2)]

            # --- pass-1 psums (qk pool created last, closes first: LIFO) ---
            p1stack = ctx.enter_context(contextlib.ExitStack())
            qkstack = ctx.enter_context(contextlib.ExitStack())
            ps_acc = p1stack.enter_context(
                tc.tile_pool(name="ps_acc", bufs=1, space="PSUM"))
            ps_qk = qkstack.enter_context(
                tc.tile_pool(name="ps_qk", bufs=3, space="PSUM"))
            acc1 = ps_acc.tile([128, 512], F32, tag="acc1")
            acc2 = ps_acc.tile([128, 256], F32, tag="acc2")
            par_all = acc1[:, 0:256]
            pgq = acc1[:, 256:512]
            pgk = acc2

            def build_pads(pool, dc, name, band, dt8):
                """Load the [128, FA] padded AB|AD tile for one band in ONE
                two-range DMA from the host-baked shifted canvas."""
                P = pool.tile([128, FA], dt8, tag=f"pad{name}")
                o = (HB * band) * PW
                src = dc[:].rearrange(
                    "p (two l) -> p two l", l=LC)[:, :, o: o + TWA]
                dst = P[:][:, 0: 2 * ADB].rearrange(
                    "p (two w) -> p two w", two=2)[:, :, 0:TWA]
                nc.sync.dma_start(out=dst, in_=src)
                return P

            # band-0 pads: separate per-region DMAs (x on sync, y on the
            # gpsimd queue) so the first q-conv DR waits only on x's AB
            # region; q convs are emitted before k convs to match
            def build_pads0(dc, name, dt8, eng):
                P = pads.tile([128, FA], dt8, tag=f"pad{name}")
                src3 = dc[:].rearrange("p (two l) -> p two l", l=LC)
                ap = P[:]
                eng.dma_start(out=ap[:, 0:TWA], in_=src3[:, 0, 0:TWA])
                eng.dma_start(out=ap[:, ADB: ADB + TWA],
                              in_=src3[:, 1, 0:TWA])
                return P

            xP0 = build_pads0(pad_d["xc"], "x", E4, nc.sync)
            nc.scalar.dma_start(out=wq_sb, in_=wq_d[:])
            yP0 = build_pads0(pad_d["yc"], "y", E4, nc.gpsimd)
            nc.scalar.dma_start(out=wk_sb, in_=wk_d[:])
            nc.gpsimd.dma_start(out=v9_sb, in_=v9_d[:])
            nc.gpsimd.dma_start(out=wp_sb, in_=wp_d[:])
            nc.gpsimd.dma_start(out=wf_sb, in_=wf_d[:])
            nc.gpsimd.dma_start(out=tcol[0], in_=tc0_d[:])
            nc.gpsimd.dma_start(out=tcol[1], in_=tc1_d[:])
            nc.gpsimd.dma_start(out=ident_sb, in_=id_d[:])

            def conv_drs(P, w_sb, pt, base, start_tag):
                """3 DR matmuls accumulating one row's conv into pt [128,256]."""
                ap = P[:]
                w3 = w_sb[:].rearrange("p (s n) -> p s n", n=256)
                pairs = [
                    (base, P0D, 0),
                    (base + 2 * PW, P1D, 2),
                    (ADB + base + 2, P2D, 4),
                ]
                for j, (X, D_, ws) in enumerate(pairs):
                    nc.tensor.matmul(
                        pt,
                        lhsT=_pair_ap(ap, X, D_, 128),
                        rhs=w3[:, ws: ws + 2, :],
                        start=(j == 0),
                        stop=(j == 2),
                        perf_mode=DR,
                    )

            # ================= pass 1: q/k convs + grams =================
            pend_gram = None  # (cat tile, first, last)

            def emit_grams(cat, first, last):
                cat3 = cat[:].rearrange("p (two c) -> p two c", two=2)
                for mb in range(2):
                    qsl = cat3[:, :, 256 + 128 * mb: 256 + 128 * mb + 128]
                    ksl = cat3[:, :, 128 * mb: 128 * mb + 128]
                    nc.tensor.matmul(
                        par_all[:, bass.ds(mb * 128, 128)],
                        lhsT=qsl, rhs=ksl, start=first, stop=last,
                        perf_mode=DR, skip_group_check=True)
                    nc.tensor.matmul(
                        pgq[:, bass.ds(mb * 128, 128)],
                        lhsT=qsl, rhs=qsl, start=first, stop=last,
                        perf_mode=DR, skip_group_check=True)
                    nc.tensor.matmul(
                        pgk[:, bass.ds(mb * 128, 128)],
                        lhsT=ksl, rhs=ksl, start=first, stop=last,
                        perf_mode=DR, skip_group_check=True)

            # persistent fus-input slabs, preloaded during pass 1 (the DMA
            # device has slack there; phase 2 then only moves pads + output)
            fuspool = ctx.enter_context(tc.tile_pool(name="fusp", bufs=1))
            xy_slab = fuspool.tile([128, N], BF16, tag="xyslab", name="xyslab")
            z_slab = fuspool.tile([64, N], BF16, tag="zslab", name="zslab")

            for b in range(NB):
                xP = xP0 if b == 0 else build_pads(
                    pads, pad_d["xc"], "x", b, E4)
                yP = yP0 if b == 0 else build_pads(
                    pads, pad_d["yc"], "y", b, E4)
                n0 = b * (N // NB)
                nc.gpsimd.dma_start(
                    out=xy_slab[:, bass.ds(n0, N // NB)],
                    in_=xyd[:, bass.ds(n0, N // NB)])
                nc.gpsimd.dma_start(
                    out=z_slab[:, bass.ds(n0, N // NB)],
                    in_=zd[:, bass.ds(n0, N // NB)])
                for hl2 in range(HB // 2):
                    pqk = ps_qk.tile([128, 1024], F32, tag="pqk")
                    for half in range(2):  # even/odd row of the pair
                        base = (2 * hl2 + half) * PW
                        o = 512 * half
                        conv_drs(xP, wq_sb, pqk[:, o + 256: o + 512], base, "q")
                        conv_drs(yP, wk_sb, pqk[:, o: o + 256], base, "k")
                    cat = qkp.tile([128, 1024], E4, tag="cat")
                    nc.scalar.copy(cat[:, 0:512], pqk[:, 0:512])
                    nc.vector.tensor_copy(cat[:, 512:1024], pqk[:, 512:1024])
                    if pend_gram is not None:
                        emit_grams(*pend_gram)
                    pg = 8 * b + hl2
                    pend_gram = (cat, pg == 0, pg == 63)
            emit_grams(*pend_gram)
            qkstack.close()
            # prefetch z band-0 pads early (DMA only; overlaps pass-1 tail)
            zP0 = build_pads(pads, pad_d["zc"], "z", 0, E5)

            # --- phase-2 pools + prefill: fus matmuls run on the PE while
            # the softmax chain occupies ACT/DVE ---
            p2p = ctx.enter_context(tc.tile_pool(name="p2p", bufs=4))
            ps_o = ctx.enter_context(
                tc.tile_pool(name="ps_o", bufs=6, space="PSUM"))

            def p2_fus(g):
                n0 = 512 * g
                pos = []
                for mb in range(2):
                    po = ps_o.tile([128, 512], F32, tag="po")
                    nc.tensor.matmul(
                        po, lhsT=wf_sb[:, bass.ds(mb * 128, 128)],
                        rhs=xy_slab[:, bass.ds(n0, 512)],
                        start=True, stop=False)
                    nc.tensor.matmul(
                        po, lhsT=wf_sb[0:64, bass.ds(256 + mb * 128, 128)],
                        rhs=z_slab[:, bass.ds(n0, 512)],
                        start=False, stop=False)
                    pos.append(po)
                return pos

            # slot 4 (pair tile 2, first half) is zeros; k-chunk 4 -> slot 5
            nc.gpsimd.memset(w2t_t[2][:, 0:256], 0.0)
            nc.gpsimd.memset(w2t_t[2][64:128, 256:512], 0.0)
            prefill = {g: p2_fus(g) for g in range(3)}

            # ================= softmax on per-head [32,32] =================
            ar_sb = [smallp.tile([128, 128], F32, tag=f"arsb{mb}",
                                 name=f"arsb{mb}") for mb in range(2)]
            nc.scalar.copy(ar_sb[0], par_all[:, 0:128])
            nc.scalar.copy(ar_sb[1], par_all[:, 128:256])
            bd8 = [smallp.tile([128, 128], E4, tag=f"bd{mb}",
                               name=f"bdiag{mb}") for mb in range(2)]
            for mb in range(2):
                scr = smallp.tile([128, 128], F32, tag="scr")
                rnq_c = smallp.tile([128, 1], F32, tag="rnq")
                rnk_c = smallp.tile([128, 1], F32, tag="rnk")
                for g_ps, dst in ((pgq, rnq_c), (pgk, rnk_c)):
                    ssum = smallp.tile([128, 1], F32, tag="ssum")
                    nc.vector.tensor_mul(
                        scr, g_ps[:, bass.ds(mb * 128, 128)], ident_sb)
                    nc.vector.reduce_sum(
                        out=ssum, in_=scr, axis=mybir.AxisListType.X)
                    nc.scalar.sqrt(ssum, ssum)
                    nc.vector.tensor_scalar_max(ssum, ssum, 1e-12)
                    nc.vector.reciprocal(dst, ssum)
                rnqt = smallp.tile([128, 1], F32, tag="rnqt")
                nc.vector.tensor_mul(rnqt, rnq_c, tcol[mb])

                hd = smallp.tile([128, 32], F32, tag="hd")
                for i in range(4):
                    nc.vector.tensor_copy(
                        hd[32 * i: 32 * (i + 1), :],
                        ar_sb[mb][32 * i: 32 * (i + 1), bass.ds(32 * i, 32)],
                    )
                hds = smallp.tile([128, 32], F32, tag="hds")
                nc.scalar.activation(
                    hds, hd, mybir.ActivationFunctionType.Copy,
                    bias=0.0, scale=rnqt)
                hdT = smallp.tile([128, 32], F32, tag="hdT")
                nc.vector.transpose(hdT, hds)
                hdTs = smallp.tile([128, 32], F32, tag="hdTs")
                nc.scalar.activation(
                    hdTs, hdT, mybir.ActivationFunctionType.Copy,
                    bias=0.0, scale=rnk_c)
                hd3 = smallp.tile([128, 32], F32, tag="hd3")
                nc.vector.transpose(hd3, hdTs)
                nmx = smallp.tile([128, 1], F32, tag="nmx")
                nc.vector.reduce_max(
                    out=nmx, in_=hd3, axis=mybir.AxisListType.X, negate=True)
                ex = smallp.tile([128, 32], F32, tag="ex")
                nc.scalar.activation(
                    ex, hd3, mybir.ActivationFunctionType.Exp,
                    bias=nmx, scale=1.0)
                sm = smallp.tile([128, 1], F32, tag="sm")
                nc.vector.reduce_sum(out=sm, in_=ex, axis=mybir.AxisListType.X)
                rsm = smallp.tile([128, 1], F32, tag="rsm")
                nc.vector.reciprocal(rsm, sm)
                rsm32 = smallp.tile([128, 1], F32, tag="rsm32")
                nc.scalar.activation(
                    rsm32, rsm, mybir.ActivationFunctionType.Copy,
                    bias=0.0, scale=AT_SCALE)
                Pt = smallp.tile([128, 32], F32, tag="Pt")
                nc.scalar.activation(
                    Pt, ex, mybir.ActivationFunctionType.Copy,
                    bias=0.0, scale=rsm32)
                PtT = smallp.tile([128, 32], F32, tag="PtT")
                nc.vector.transpose(PtT, Pt)
                nc.gpsimd.memset(bd8[mb], 0.0)
                for i in range(4):
                    nc.vector.tensor_copy(
                        bd8[mb][32 * i: 32 * (i + 1), bass.ds(32 * i, 32)],
                        PtT[32 * i: 32 * (i + 1), :],
                    )

            # ================= W2T build (reuses acc psum banks) =======
            # W1 = (attn*32) @ V9 into the dead gram psums: acc1 holds
            # cols 0:512, acc2[:, 192:256] the 64-tail
            w1_sb = smallp.tile([128, 2 * 576], E4, tag="w1sb", name="w1sb")
            for mb in range(2):
                vsl = v9_sb[:, 576 * mb: 576 * mb + 576]
                nc.tensor.matmul(acc1, lhsT=bd8[mb],
                                 rhs=vsl[:, 0:512], start=True, stop=True,
                                 skip_group_check=True)
                nc.tensor.matmul(acc2[:, 192:256], lhsT=bd8[mb],
                                 rhs=vsl[:, 512:576], start=True, stop=True,
                                 skip_group_check=True)
                nc.scalar.copy(w1_sb[:, 576 * mb: 576 * mb + 512], acc1)
                nc.vector.tensor_copy(
                    w1_sb[:, 576 * mb + 512: 576 * mb + 576],
                    acc2[:, 192:256])
            w13 = w1_sb[:].rearrange("p (two k) -> p two k", two=2)
            wp3 = wp_sb[:].rearrange("p (two n) -> p two n", two=2)
            for j in range(5):
                kw = 128 if j < 4 else 64
                wbuf = (acc2 if j % 2 == 0 else acc1)[0:kw, 0:256]
                nc.tensor.matmul(
                    wbuf,
                    lhsT=w13[:, :, 128 * j: 128 * j + kw],
                    rhs=wp3, start=True, stop=True, perf_mode=DR,
                    skip_group_check=True)
                dstt = w2t_t[j // 2] if j < 4 else w2t_t[2]
                dsts = (j % 2) if j < 4 else 1
                nc.scalar.activation(
                    dstt[0:kw, dsts * 256: (dsts + 1) * 256], wbuf,
                    mybir.ActivationFunctionType.Copy,
                    bias=0.0, scale=float(W2_COPY_SCALE))

            # ================= phase 2: fus + W2T@z9 =================
            w2t3 = [t[:].rearrange("p (s n) -> p s n", n=256) for t in w2t_t]
            od3 = od[:].rearrange("(two p) c -> p two c", two=2)

            def p2_attn_out(g, zap, pos):
                n0 = 512 * g
                cc = g % 4
                o2 = p2p.tile([128, 1024], BF16, tag="o2", name="o2")
                for mb in range(2):
                    po = pos[mb]
                    for r in range(4):
                        base = (4 * cc + r) * PW
                        pairs = [
                            (base, P0D, 0),
                            (base + 2 * PW, P1D, 1),
                            (ADB + base + 2, P2D, 2),
                        ]
                        for j, (X, D_, wk_) in enumerate(pairs):
                            nc.tensor.matmul(
                                po[:, 128 * r: 128 * r + 128],
                                lhsT=w2t3[wk_][:, :,
                                               128 * mb: 128 * mb + 128],
                                rhs=_pair_ap(zap, X, D_, 128),
                                start=False,
                                stop=(r == 3 and j == 2),
                                perf_mode=DR,
                                skip_group_check=True)
                    if mb == 0:
                        nc.scalar.copy(o2[:, 0:512], po)
                    else:
                        nc.vector.tensor_copy(o2[:, 512:1024], po)
                # one DMA for both halves: dst rows (p, p+128), cols n0..+512
                dst = od3[:, :, bass.ds(n0, 512)]
                src = o2[:].rearrange("p (two c) -> p two c", two=2)
                nc.sync.dma_start(out=dst, in_=src)

            for bz in range(NB):
                zP = zP0 if bz == 0 else build_pads(
                    pads, pad_d["zc"], "z", bz, E5)
                zap = zP[:]
                for cc in range(4):
                    g = 4 * bz + cc
                    pos = prefill.pop(g, None) or p2_fus(g)
                    p2_attn_out(g, zap, pos)

    _split_excess_waits(nc)
    return nc


def _prep_weights(inputs):
    wq = _merge_w(np.asarray(inputs["Wq"], np.float32),
                  np.asarray(inputs["Wq_dw"], np.float32), QK_SCALE)
    wk = _merge_w(np.asarray(inputs["Wk"], np.float32),
                  np.asarray(inputs["Wk_dw"], np.float32), QK_SCALE)
    v9 = _merge_v9(np.asarray(inputs["Wv"], np.float32),
                   np.asarray(inputs["Wv_dw"], np.float32), V9_SCALE)

    wproj = np.asarray(inputs["Wproj"], np.float32)[:, :, 0, 0]  # [256,256]
    # WprojN [c, o] mb tiles side by side: [128, 512]
    wprojN = np.zeros((128, 512), np.float32)
    wprojN[:, 0:256] = wproj[:, 0:128].T * PR_SCALE
    wprojN[:, 256:512] = wproj[:, 128:256].T * PR_SCALE

    wfus = np.asarray(inputs["Wfus"], np.float32)[:, :, 0, 0]  # [256, 192]
    wfusT = np.zeros((128, 512), np.float32)
    wfusT[:, 0:256] = wfus[:, 0:128].T          # x,y rows
    wfusT[0:64, 256:512] = wfus[:, 128:192].T   # z rows

    temp = np.asarray(inputs["temperature"], np.float32).reshape(HEADS)
    tfull = np.repeat(temp, 32).astype(np.float32)
    temp_cols = [tfull[0:128].reshape(128, 1), tfull[128:256].reshape(128, 1)]
    return wq, wk, v9, wprojN, wfusT, temp_cols


def _canvas(img, np8):
    """img [64, 128, 128] fp32 -> [128, 2*130*PW] canvas in np8: cols
    [0, LC) = AB (parts 0:64 = A padded image at pitch PW, 64:128 =
    B = A<<1col), cols [LC, 2LC) = AD (A | D = A<<1row)."""
    LC = 130 * PW
    A = np.zeros((64, 130, PW), np.float32)
    A[:, 1:129, 1:129] = img
    Af = A.reshape(64, LC)
    ext = np.zeros((64, LC + PW + 8), np.float32)
    ext[:, :LC] = Af
    out = np.zeros((128, 2 * LC), np.float32)
    out[0:64, 0:LC] = Af
    out[64:128, 0:LC] = ext[:, 1: LC + 1]
    out[0:64, LC:] = Af
    out[64:128, LC:] = ext[:, PW: LC + PW]
    return out.astype(np8)


def kernel(**inputs):
    x = np.asarray(inputs["x"], np.float32)
    y = np.asarray(inputs["y"], np.float32)
    z = np.asarray(inputs["z"], np.float32)
    B = x.shape[0]
    assert B == 8

    nc = _build_nc(*_prep_weights(inputs))

    in_maps = []
    for i in range(B):
        xi = x[i].reshape(C, N)
        yi = y[i].reshape(C, N)
        zi = z[i].reshape(C, N)
        in_maps.append({
            "xy": _bf(np.concatenate([xi, yi], axis=0)),
            "z": _bf(zi),
            "xc": _canvas(x[i], NP_E4),
            "yc": _canvas(y[i], NP_E4),
            "zc": _canvas(z[i] * Z8_SCALE, NP_E5),
        })
    res = run_bass_kernel_spmd(nc, in_maps, list(range(8)))
    out = np.stack(
        [np.asarray(res.results[i]["out"]).astype(np.float32).reshape(DIM, H, W)
         for i in range(B)]
    )
    return out
